# revision 4
# baseline (speedup 1.0000x reference)
"""Multi-head self-attention (B=4, T=2048, C=768, H=12) on 8 trn2 NeuronCores. v9.

Sharding: core c -> batch b=c//2, head-group g=c%2 (6 heads each).
Host sums the 2 partials per batch and adds the bias.

Design:
  - bf16 operands on the PE (FWL weight loads, hidden LDWEIGHTS, 2x DVE modes).
  - head-PAIR row-tiled scores: heads (2m, 2m+1) occupy partition halves of
    qT[m]/kT[m]; score matmuls at tile_position (0,0)/(64,0) run concurrently.
  - v computed directly in [t, d] layout into per-(pair, chunk, head) v_aug
    tiles; a ones column rides along so softmax denominators fall out of the
    ctx matmul.
  - exp mostly on ACT ([128,1024] chunks, bf16 out); ~17% of chunks offloaded
    to DVE via a Schraudolph bf16 exp (tensor_scalar -> int16 bits == bf16).
  - first block's scores+exp are emitted before the v projections (deep pt
    pool buffers 16 chunks) so the ACT stream starts ~25us in; its ctx half
    runs after v lands. Later pairs' projections are emitted mid-attention to
    fill PE gaps.
  - softmax normalize: one [65,1024] copy releases the cps bank; recip +
    gpsimd partition_broadcast + scales run from SBUF off the critical path.

PSUM: lead pool 2 (transposes+projections share slots) + sps 2x2 + cps 2 = 8.
"""
import sys
import os

sys.path.insert(0, "/opt/trn_rl_repo")

import numpy as np

P = 128
T = 2048
C = 768
HD = 384          # per-core head columns (6 heads x 64)
D = 64
NT = T // P       # 16 key chunks of 128
KC = C // P       # 6 contraction chunks for C
MC = HD // P      # 3 head pairs
QB = 512          # query block
NQ = T // QB      # 4 query blocks

EXP_A = 128 * 0.125 * float(np.log2(np.e))   # 23.083120654223414
EXP_B = 128 * 127 - 5.5                      # 16250.5 (Schraudolph bias, bf16)

_cache = {}


def _build(repeat=1):
    import concourse.bacc as bacc
    import concourse.mybir as mybir
    import concourse.tile as tile
    from concourse.masks import make_identity
    from contextlib import ExitStack

    F32 = mybir.dt.float32
    BF16 = mybir.dt.bfloat16
    I16 = mybir.dt.int16
    AF = mybir.ActivationFunctionType
    ALU = mybir.AluOpType

    nc = bacc.Bacc("TRN2", target_bir_lowering=False, debug=False)
    x = nc.dram_tensor("x", [T, C], F32, kind="ExternalInput").ap()
    wq = nc.dram_tensor("wq", [C, HD], F32, kind="ExternalInput").ap()
    wk = nc.dram_tensor("wk", [C, HD], F32, kind="ExternalInput").ap()
    wv = nc.dram_tensor("wv", [C, HD], F32, kind="ExternalInput").ap()
    wo = nc.dram_tensor("wo", [HD, C], F32, kind="ExternalInput").ap()
    out = nc.dram_tensor("out", [T, C], F32, kind="ExternalOutput").ap()

    def emit(pfx, tc, pools):
        ident_bf, big, wrp, vap, work, outp, norm = pools

        xt = [big.tile([P, T], BF16, name=f"{pfx}xt{kc}", tag="big2048") for kc in range(KC)]
        qT = [big.tile([P, T], BF16, name=f"{pfx}qT{m}", tag="big2048") for m in range(MC)]
        kT = [big.tile([P, T], BF16, name=f"{pfx}kT{m}", tag="big2048") for m in range(MC)]
        ctxT = [big.tile([P, T], BF16, name=f"{pfx}ctxT{m}", tag="big2048") for m in range(MC)]
        va = [[[vap.tile([P, D + 1], BF16, name=f"{pfx}va{m}_{t}_{g}", tag=f"va{m}_{t}_{g}")
                for g in range(2)] for t in range(NT)] for m in range(MC)]

        w_b = {}
        wo_b = []

        def emit_weight_loads(wstage, names):
            for nm, src in names:
                if nm == "o":
                    for m in range(MC):
                        st = wstage.tile([P, C], F32, name=f"{pfx}wst_o{m}", tag="wsto")
                        nc.sync.dma_start(st[:], wo[P * m:P * (m + 1), :])
                        t_b = wrp.tile([P, C], BF16, name=f"{pfx}wo_{m}", tag=f"wo_{m}")
                        nc.vector.tensor_copy(t_b[:], st[:])
                        wo_b.append(t_b)
                    continue
                for kc in range(KC):
                    st = wstage.tile([P, HD], F32, name=f"{pfx}wst_{nm}{kc}", tag="wst")
                    nc.sync.dma_start(st[:], src[P * kc:P * (kc + 1), :])
                    t_b = wrp.tile([P, HD], BF16, name=f"{pfx}w_{nm}{kc}", tag=f"w_{nm}{kc}")
                    nc.vector.tensor_copy(t_b[:], st[:])
                    w_b[nm, kc] = t_b

        def emit_xt_group(tq, xrp, xbp, lead):
            xbs = []
            for i in range(4):
                t_i = 4 * tq + i
                xr = xrp.tile([P, C], F32, name=f"{pfx}xr{t_i}", tag="xr")
                nc.sync.dma_start(xr[:], x[P * t_i:P * (t_i + 1), :])
                xb = xbp.tile([P, C], BF16, name=f"{pfx}xb{t_i}", tag="xb")
                nc.vector.tensor_copy(xb[:], xr[:])
                xbs.append(xb)
            for kc in range(KC):
                tp = lead.tile([P, 512], BF16, name=f"{pfx}tp_{tq}_{kc}", tag="lead")
                for i in range(4):
                    nc.tensor.transpose(tp[:, P * i:P * (i + 1)],
                                        xbs[i][:, P * kc:P * (kc + 1)], ident_bf[:])
                nc.vector.tensor_copy(xt[kc][:, 512 * tq:512 * (tq + 1)], tp[:])

        def emit_proj_block(nm, m, n, lead):
            dest = qT if nm == "q" else kT
            ps = lead.tile([P, 512], F32, name=f"{pfx}ps_{nm}{m}{n}", tag="lead")
            for kc in range(KC):
                nc.tensor.matmul(
                    ps[:],
                    w_b[nm, kc][:, P * m:P * (m + 1)],
                    xt[kc][:, 512 * n:512 * (n + 1)],
                    start=(kc == 0), stop=(kc == KC - 1),
                )
            nc.vector.tensor_copy(dest[m][:, 512 * n:512 * (n + 1)], ps[:])

        def emit_v_chunk(t_i, lead):
            pv = lead.tile([P, HD], F32, name=f"{pfx}pv{t_i}", tag="lead")
            for kc in range(KC):
                nc.tensor.matmul(
                    pv[:],
                    xt[kc][:, P * t_i:P * (t_i + 1)],
                    w_b["v", kc][:],
                    start=(kc == 0), stop=(kc == KC - 1),
                )
            for m in range(MC):
                for g in range(2):
                    vt = va[m][t_i][g]
                    nc.vector.tensor_copy(vt[:, 0:D], pv[:, P * m + D * g:P * m + D * (g + 1)])
                    nc.gpsimd.memset(vt[:, D:D + 1], 1.0)

        def scores_exp(m, u, j, spsp):
            q0 = QB * u
            sps = spsp.tile([P, 2 * QB], F32, name=f"{pfx}sps{m}{u}{j}", tag="sps")
            nc.tensor.matmul(sps[:, 0:QB],
                             kT[m][0:D, P * j:P * (j + 1)],
                             qT[m][0:D, q0:q0 + QB],
                             start=True, stop=True, tile_position=(0, 0))
            nc.tensor.matmul(sps[:, QB:2 * QB],
                             kT[m][D:P, P * j:P * (j + 1)],
                             qT[m][D:P, q0:q0 + QB],
                             start=True, stop=True, tile_position=(64, 0))
            pt = work.tile([P, 2 * QB], BF16, name=f"{pfx}pt{m}{u}{j}", tag="pt")
            if (m, u) != (0, 0) and (u * NT + j) % 4 == 3 and 3 <= j <= 12:
                nc.vector.tensor_scalar(pt[:].bitcast(I16), sps[:],
                                        EXP_A, EXP_B, ALU.mult, ALU.add)
            else:
                nc.scalar.activation(pt[:], sps[:], AF.Exp, scale=float(D) ** -0.5)
            return pt

        def ctx_accum(m, u, j, pt, cps):
            for g in range(2):
                nc.tensor.matmul(cps[:, QB * g:QB * (g + 1)],
                                 va[m][j][g][:],
                                 pt[:, QB * g:QB * (g + 1)],
                                 start=(j == 0), stop=(j == NT - 1))

        def normalize(m, u, cps):
            q0 = QB * u
            cu = norm.tile([D + 1, 2 * QB], F32, name=f"{pfx}cu{m}{u}", tag="cu")
            nc.vector.tensor_copy(cu[:], cps[:])
            s_sb = norm.tile([1, 2 * QB], F32, name=f"{pfx}ssb{m}{u}", tag="ssb")
            nc.vector.tensor_copy(s_sb[:], cu[D:D + 1, :])
            rr = norm.tile([1, 2 * QB], F32, name=f"{pfx}rr{m}{u}", tag="rr")
            nc.vector.reciprocal_approx_fast(rr[:], s_sb[:])
            rb = norm.tile([D, 2 * QB], F32, name=f"{pfx}rb{m}{u}", tag="rb")
            nc.gpsimd.partition_broadcast(rb[:], rr[:])
            nc.vector.tensor_mul(ctxT[m][0:D, q0:q0 + QB], cu[0:D, 0:QB], rb[:, 0:QB])
            nc.vector.tensor_mul(ctxT[m][D:P, q0:q0 + QB], cu[0:D, QB:2 * QB], rb[:, QB:2 * QB])

        def attn_block(m, u, spsp, cpsp):
            cps = cpsp.tile([D + 1, 2 * QB], F32, name=f"{pfx}cps{m}_{u}", tag="cps")
            for j in range(NT):
                pt = scores_exp(m, u, j, spsp)
                ctx_accum(m, u, j, pt, cps)
            normalize(m, u, cps)

        def outproj(u, lead):
            for t_i in range(4 * u, 4 * (u + 1)):
                psA = lead.tile([P, 512], F32, name=f"{pfx}psA{t_i}", tag="lead")
                psB = lead.tile([P, C - 512], F32, name=f"{pfx}psB{t_i}", tag="lead")
                for m in range(MC):
                    nc.tensor.matmul(psA[:], ctxT[m][:, P * t_i:P * (t_i + 1)],
                                     wo_b[m][:, 0:512], start=(m == 0), stop=(m == MC - 1))
                    nc.tensor.matmul(psB[:], ctxT[m][:, P * t_i:P * (t_i + 1)],
                                     wo_b[m][:, 512:C], start=(m == 0), stop=(m == MC - 1))
                ob = outp.tile([P, C], F32, name=f"{pfx}ob{t_i}", tag="ob")
                nc.vector.tensor_copy(ob[:, 0:512], psA[:])
                nc.vector.tensor_copy(ob[:, 512:C], psB[:])
                nc.sync.dma_start(out[P * t_i:P * (t_i + 1), :], ob[:])

        # ================= emission =================
        with tc.tile_pool(name=pfx + "xrp", bufs=4) as xrp, \
             tc.tile_pool(name=pfx + "xbp", bufs=5) as xbp, \
             tc.tile_pool(name=pfx + "wstage", bufs=3) as wstage, \
             tc.tile_pool(name=pfx + "lead", bufs=2, space="PSUM") as lead, \
             tc.tile_pool(name=pfx + "sps", bufs=2, space="PSUM") as spsp, \
             tc.tile_pool(name=pfx + "cps", bufs=1, space="PSUM") as cpsp:
            emit_xt_group(0, xrp, xbp, lead)
            emit_weight_loads(wstage, [("k", wk), ("q", wq), ("v", wv)])
            emit_proj_block("k", 0, 0, lead)
            emit_proj_block("q", 0, 0, lead)
            for tq in range(1, 4):
                emit_xt_group(tq, xrp, xbp, lead)
                emit_proj_block("k", 0, tq, lead)
                emit_proj_block("q", 0, tq, lead)
            # first block's scores+exp start while v is still being produced
            cps00 = cpsp.tile([D + 1, 2 * QB], F32, name=f"{pfx}cps0_0", tag="cps")
            pts00 = [scores_exp(0, 0, j, spsp) for j in range(NT)]
            for t_i in range(NT):
                emit_v_chunk(t_i, lead)
            emit_weight_loads(wstage, [("o", wo)])
            for j in range(NT):
                ctx_accum(0, 0, j, pts00[j], cps00)
            normalize(0, 0, cps00)
            attn_block(0, 1, spsp, cpsp)
            # next pair's projections fill this pair's ACT-bound gaps
            for n in range(4):
                emit_proj_block("k", 1, n, lead)
                emit_proj_block("q", 1, n, lead)
            attn_block(0, 2, spsp, cpsp)
            attn_block(0, 3, spsp, cpsp)
            attn_block(1, 0, spsp, cpsp)
            for n in range(4):
                emit_proj_block("k", 2, n, lead)
                emit_proj_block("q", 2, n, lead)
            for u in range(1, NQ):
                attn_block(1, u, spsp, cpsp)
            for u in range(NQ):
                attn_block(2, u, spsp, cpsp)
                outproj(u, lead)

    with tile.TileContext(nc) as tc, ExitStack() as ctx:
        consts = ctx.enter_context(tc.tile_pool(name="consts", bufs=1))
        ident_f32 = consts.tile([P, P], mybir.dt.float32)
        make_identity(nc, ident_f32)
        ident_bf = consts.tile([P, P], BF16)
        nc.vector.tensor_copy(ident_bf[:], ident_f32[:])

        big = ctx.enter_context(tc.tile_pool(name="big", bufs=12))
        wrp = ctx.enter_context(tc.tile_pool(name="wrp", bufs=1))
        vap = ctx.enter_context(tc.tile_pool(name="vap", bufs=1))
        work = ctx.enter_context(tc.tile_pool(name="work", bufs=20))
        outp = ctx.enter_context(tc.tile_pool(name="outp", bufs=2))
        norm = ctx.enter_context(tc.tile_pool(name="norm", bufs=2))
        pools = (ident_bf, big, wrp, vap, work, outp, norm)
        for rep in range(repeat):
            emit(f"r{rep}_", tc, pools)

    nc.compile()
    return nc


def kernel(X, Wq, Wk, Wv, Wo, bo):
    from concourse import bass_utils

    if "nc" not in _cache:
        _cache["nc"] = _build(int(os.environ.get("KERNEL_REPEAT", "1")))
    nc = _cache["nc"]

    X = np.asarray(X, dtype=np.float32)
    in_maps = []
    for c in range(8):
        b, g = divmod(c, 2)
        sl = slice(HD * g, HD * (g + 1))
        in_maps.append({
            "x": np.ascontiguousarray(X[b]),
            "wq": np.ascontiguousarray(np.asarray(Wq, np.float32)[:, sl]),
            "wk": np.ascontiguousarray(np.asarray(Wk, np.float32)[:, sl]),
            "wv": np.ascontiguousarray(np.asarray(Wv, np.float32)[:, sl]),
            "wo": np.ascontiguousarray(np.asarray(Wo, np.float32)[sl, :]),
        })
    res = bass_utils.run_bass_kernel_spmd(nc, in_maps, core_ids=list(range(8)))
    _cache["last_result"] = res
    outf = np.empty((4, T, C), np.float32)
    bo = np.asarray(bo, np.float32)
    for b in range(4):
        outf[b] = res.results[2 * b]["out"] + res.results[2 * b + 1]["out"] + bo
    return outf


# revision 5
# speedup vs baseline: 1.0358x; 1.0358x over previous
"""Multi-head self-attention (B=4, T=2048, C=768, H=12) on 8 trn2 NeuronCores. v9.

Sharding: core c -> batch b=c//2, head-group g=c%2 (6 heads each).
Host sums the 2 partials per batch and adds the bias.

Design:
  - bf16 operands on the PE (FWL weight loads, hidden LDWEIGHTS, 2x DVE modes).
  - head-PAIR row-tiled scores: heads (2m, 2m+1) occupy partition halves of
    qT[m]/kT[m]; score matmuls at tile_position (0,0)/(64,0) run concurrently.
  - v computed directly in [t, d] layout into per-(pair, chunk, head) v_aug
    tiles; a ones column rides along so softmax denominators fall out of the
    ctx matmul.
  - exp mostly on ACT ([128,1024] chunks, bf16 out); ~17% of chunks offloaded
    to DVE via a Schraudolph bf16 exp (tensor_scalar -> int16 bits == bf16).
  - first block's scores+exp are emitted before the v projections (deep pt
    pool buffers 16 chunks) so the ACT stream starts ~25us in; its ctx half
    runs after v lands. Later pairs' projections are emitted mid-attention to
    fill PE gaps.
  - softmax normalize: one [65,1024] copy releases the cps bank; recip +
    gpsimd partition_broadcast + scales run from SBUF off the critical path.

PSUM: lead pool 2 (transposes+projections share slots) + sps 2x2 + cps 2 = 8.
"""
import sys
import os

sys.path.insert(0, "/opt/trn_rl_repo")

import numpy as np

P = 128
T = 2048
C = 768
HD = 384          # per-core head columns (6 heads x 64)
D = 64
NT = T // P       # 16 key chunks of 128
KC = C // P       # 6 contraction chunks for C
MC = HD // P      # 3 head pairs
QB = 512          # query block
NQ = T // QB      # 4 query blocks

EXP_A = 128 * 0.125 * float(np.log2(np.e))   # 23.083120654223414
EXP_B = 128 * 127 - 5.5                      # 16250.5 (Schraudolph bias, bf16)

_cache = {}


def _build(repeat=1):
    import concourse.bacc as bacc
    import concourse.mybir as mybir
    import concourse.tile as tile
    from concourse.masks import make_identity
    from contextlib import ExitStack

    F32 = mybir.dt.float32
    BF16 = mybir.dt.bfloat16
    I16 = mybir.dt.int16
    AF = mybir.ActivationFunctionType
    ALU = mybir.AluOpType

    nc = bacc.Bacc("TRN2", target_bir_lowering=False, debug=False)
    x = nc.dram_tensor("x", [T, C], F32, kind="ExternalInput").ap()
    wq = nc.dram_tensor("wq", [C, HD], F32, kind="ExternalInput").ap()
    wk = nc.dram_tensor("wk", [C, HD], F32, kind="ExternalInput").ap()
    wv = nc.dram_tensor("wv", [C, HD], F32, kind="ExternalInput").ap()
    wo = nc.dram_tensor("wo", [HD, C], F32, kind="ExternalInput").ap()
    out = nc.dram_tensor("out", [T, C], F32, kind="ExternalOutput").ap()

    def emit(pfx, tc, pools):
        ident_bf, big, wrp, vap, work, outp, norm = pools

        xt = [big.tile([P, T], BF16, name=f"{pfx}xt{kc}", tag="big2048") for kc in range(KC)]
        qT = [big.tile([P, T], BF16, name=f"{pfx}qT{m}", tag="big2048") for m in range(MC)]
        kT = [big.tile([P, T], BF16, name=f"{pfx}kT{m}", tag="big2048") for m in range(MC)]
        ctxT = [big.tile([P, T], BF16, name=f"{pfx}ctxT{m}", tag="big2048") for m in range(MC)]
        va = [[[vap.tile([P, D + 1], BF16, name=f"{pfx}va{m}_{t}_{g}", tag=f"va{m}_{t}_{g}")
                for g in range(2)] for t in range(NT)] for m in range(MC)]

        w_b = {}
        wo_b = []

        def emit_weight_loads(wstage, names):
            for nm, src in names:
                if nm == "o":
                    for m in range(MC):
                        st = wstage.tile([P, C], F32, name=f"{pfx}wst_o{m}", tag="wsto")
                        nc.sync.dma_start(st[:], wo[P * m:P * (m + 1), :])
                        t_b = wrp.tile([P, C], BF16, name=f"{pfx}wo_{m}", tag=f"wo_{m}")
                        nc.vector.tensor_copy(t_b[:], st[:])
                        wo_b.append(t_b)
                    continue
                for kc in range(KC):
                    st = wstage.tile([P, HD], F32, name=f"{pfx}wst_{nm}{kc}", tag="wst")
                    nc.sync.dma_start(st[:], src[P * kc:P * (kc + 1), :])
                    t_b = wrp.tile([P, HD], BF16, name=f"{pfx}w_{nm}{kc}", tag=f"w_{nm}{kc}")
                    nc.vector.tensor_copy(t_b[:], st[:])
                    w_b[nm, kc] = t_b

        def emit_xt_group(tq, xrp, xbp, lead):
            xbs = []
            for i in range(4):
                t_i = 4 * tq + i
                xr = xrp.tile([P, C], F32, name=f"{pfx}xr{t_i}", tag="xr")
                nc.sync.dma_start(xr[:], x[P * t_i:P * (t_i + 1), :])
                xb = xbp.tile([P, C], BF16, name=f"{pfx}xb{t_i}", tag="xb")
                nc.vector.tensor_copy(xb[:], xr[:])
                xbs.append(xb)
            for kc in range(KC):
                tp = lead.tile([P, 512], BF16, name=f"{pfx}tp_{tq}_{kc}", tag="lead")
                for i in range(4):
                    nc.tensor.transpose(tp[:, P * i:P * (i + 1)],
                                        xbs[i][:, P * kc:P * (kc + 1)], ident_bf[:])
                nc.vector.tensor_copy(xt[kc][:, 512 * tq:512 * (tq + 1)], tp[:])

        def emit_proj_block(nm, m, n, lead):
            dest = qT if nm == "q" else kT
            ps = lead.tile([P, 512], F32, name=f"{pfx}ps_{nm}{m}{n}", tag="lead")
            for kc in range(KC):
                nc.tensor.matmul(
                    ps[:],
                    w_b[nm, kc][:, P * m:P * (m + 1)],
                    xt[kc][:, 512 * n:512 * (n + 1)],
                    start=(kc == 0), stop=(kc == KC - 1),
                )
            nc.vector.tensor_copy(dest[m][:, 512 * n:512 * (n + 1)], ps[:])

        def emit_v_chunk(t_i, lead):
            pv = lead.tile([P, HD], F32, name=f"{pfx}pv{t_i}", tag="lead")
            for kc in range(KC):
                nc.tensor.matmul(
                    pv[:],
                    xt[kc][:, P * t_i:P * (t_i + 1)],
                    w_b["v", kc][:],
                    start=(kc == 0), stop=(kc == KC - 1),
                )
            for m in range(MC):
                for g in range(2):
                    vt = va[m][t_i][g]
                    nc.vector.tensor_copy(vt[:, 0:D], pv[:, P * m + D * g:P * m + D * (g + 1)])
                    nc.gpsimd.memset(vt[:, D:D + 1], 1.0)

        def scores_exp(m, u, j, spsp):
            q0 = QB * u
            sps = spsp.tile([P, 2 * QB], F32, name=f"{pfx}sps{m}{u}{j}", tag="sps")
            nc.tensor.matmul(sps[:, 0:QB],
                             kT[m][0:D, P * j:P * (j + 1)],
                             qT[m][0:D, q0:q0 + QB],
                             start=True, stop=True, tile_position=(0, 0))
            nc.tensor.matmul(sps[:, QB:2 * QB],
                             kT[m][D:P, P * j:P * (j + 1)],
                             qT[m][D:P, q0:q0 + QB],
                             start=True, stop=True, tile_position=(64, 0))
            pt = work.tile([P, 2 * QB], BF16, name=f"{pfx}pt{m}{u}{j}", tag="pt")
            if (m, u) != (0, 0) and (u * NT + j) % 4 == 3 and 3 <= j <= 12:
                nc.vector.tensor_scalar(pt[:].bitcast(I16), sps[:],
                                        EXP_A, EXP_B, ALU.mult, ALU.add)
            else:
                nc.scalar.activation(pt[:], sps[:], AF.Exp, scale=float(D) ** -0.5)
            return pt

        def ctx_accum(m, u, j, pt, cps):
            for g in range(2):
                nc.tensor.matmul(cps[:, QB * g:QB * (g + 1)],
                                 va[m][j][g][:],
                                 pt[:, QB * g:QB * (g + 1)],
                                 start=(j == 0), stop=(j == NT - 1))

        def normalize(m, u, cps):
            q0 = QB * u
            cu = norm.tile([D + 1, 2 * QB], F32, name=f"{pfx}cu{m}{u}", tag="cu")
            nc.vector.tensor_copy(cu[:], cps[:])
            s_sb = norm.tile([1, 2 * QB], F32, name=f"{pfx}ssb{m}{u}", tag="ssb")
            nc.vector.tensor_copy(s_sb[:], cu[D:D + 1, :])
            rr = norm.tile([1, 2 * QB], F32, name=f"{pfx}rr{m}{u}", tag="rr")
            nc.vector.reciprocal_approx_fast(rr[:], s_sb[:])
            rb = norm.tile([D, 2 * QB], F32, name=f"{pfx}rb{m}{u}", tag="rb")
            nc.gpsimd.partition_broadcast(rb[:], rr[:])
            nc.vector.tensor_mul(ctxT[m][0:D, q0:q0 + QB], cu[0:D, 0:QB], rb[:, 0:QB])
            nc.vector.tensor_mul(ctxT[m][D:P, q0:q0 + QB], cu[0:D, QB:2 * QB], rb[:, QB:2 * QB])

        def attn_block(m, u, spsp, cpsp):
            cps = cpsp.tile([D + 1, 2 * QB], F32, name=f"{pfx}cps{m}_{u}", tag="cps")
            for j in range(NT):
                pt = scores_exp(m, u, j, spsp)
                ctx_accum(m, u, j, pt, cps)
            normalize(m, u, cps)

        def outproj(u, lead):
            for t_i in range(4 * u, 4 * (u + 1)):
                psA = lead.tile([P, 512], F32, name=f"{pfx}psA{t_i}", tag="lead")
                psB = lead.tile([P, C - 512], F32, name=f"{pfx}psB{t_i}", tag="lead")
                for m in range(MC):
                    nc.tensor.matmul(psA[:], ctxT[m][:, P * t_i:P * (t_i + 1)],
                                     wo_b[m][:, 0:512], start=(m == 0), stop=(m == MC - 1))
                    nc.tensor.matmul(psB[:], ctxT[m][:, P * t_i:P * (t_i + 1)],
                                     wo_b[m][:, 512:C], start=(m == 0), stop=(m == MC - 1))
                ob = outp.tile([P, C], F32, name=f"{pfx}ob{t_i}", tag="ob")
                nc.vector.tensor_copy(ob[:, 0:512], psA[:])
                nc.vector.tensor_copy(ob[:, 512:C], psB[:])
                nc.sync.dma_start(out[P * t_i:P * (t_i + 1), :], ob[:])

        # ================= emission =================
        with tc.tile_pool(name=pfx + "xrp", bufs=4) as xrp, \
             tc.tile_pool(name=pfx + "xbp", bufs=5) as xbp, \
             tc.tile_pool(name=pfx + "wstage", bufs=3) as wstage, \
             tc.tile_pool(name=pfx + "lead", bufs=2, space="PSUM") as lead, \
             tc.tile_pool(name=pfx + "sps", bufs=2, space="PSUM") as spsp, \
             tc.tile_pool(name=pfx + "cps", bufs=1, space="PSUM") as cpsp:
            # first block's scores+exp interleave with the lead: exp stream
            # starts as soon as kT[0] n=0 / qT[0] n=0 land (~13us)
            emit_xt_group(0, xrp, xbp, lead)
            emit_weight_loads(wstage, [("k", wk), ("q", wq), ("v", wv)])
            cps00 = cpsp.tile([D + 1, 2 * QB], F32, name=f"{pfx}cps0_0", tag="cps")
            pts00 = []
            emit_proj_block("k", 0, 0, lead)
            emit_proj_block("q", 0, 0, lead)
            for j in range(4):
                pts00.append(scores_exp(0, 0, j, spsp))
            for tq in range(1, 4):
                emit_xt_group(tq, xrp, xbp, lead)
                emit_proj_block("k", 0, tq, lead)
                emit_proj_block("q", 0, tq, lead)
                for j in range(4 * tq, 4 * (tq + 1)):
                    pts00.append(scores_exp(0, 0, j, spsp))
            for t_i in range(NT):
                emit_v_chunk(t_i, lead)
            emit_weight_loads(wstage, [("o", wo)])
            for j in range(NT):
                ctx_accum(0, 0, j, pts00[j], cps00)
            normalize(0, 0, cps00)
            attn_block(0, 1, spsp, cpsp)
            attn_block(0, 2, spsp, cpsp)
            attn_block(0, 3, spsp, cpsp)
            # next pair's projections sit just before their consumers so they
            # only fill attention stalls instead of preempting the exp feed
            for n in range(4):
                emit_proj_block("k", 1, n, lead)
                emit_proj_block("q", 1, n, lead)
            for u in range(NQ):
                attn_block(1, u, spsp, cpsp)
                if u == NQ - 1:
                    for n in range(4):
                        emit_proj_block("k", 2, n, lead)
                        emit_proj_block("q", 2, n, lead)
            for u in range(NQ):
                attn_block(2, u, spsp, cpsp)
                outproj(u, lead)

    with tile.TileContext(nc) as tc, ExitStack() as ctx:
        consts = ctx.enter_context(tc.tile_pool(name="consts", bufs=1))
        ident_f32 = consts.tile([P, P], mybir.dt.float32)
        make_identity(nc, ident_f32)
        ident_bf = consts.tile([P, P], BF16)
        nc.vector.tensor_copy(ident_bf[:], ident_f32[:])

        big = ctx.enter_context(tc.tile_pool(name="big", bufs=12))
        wrp = ctx.enter_context(tc.tile_pool(name="wrp", bufs=1))
        vap = ctx.enter_context(tc.tile_pool(name="vap", bufs=1))
        work = ctx.enter_context(tc.tile_pool(name="work", bufs=20))
        outp = ctx.enter_context(tc.tile_pool(name="outp", bufs=2))
        norm = ctx.enter_context(tc.tile_pool(name="norm", bufs=2))
        pools = (ident_bf, big, wrp, vap, work, outp, norm)
        for rep in range(repeat):
            emit(f"r{rep}_", tc, pools)

    nc.compile()
    return nc


def kernel(X, Wq, Wk, Wv, Wo, bo):
    from concourse import bass_utils

    if "nc" not in _cache:
        _cache["nc"] = _build(int(os.environ.get("KERNEL_REPEAT", "1")))
    nc = _cache["nc"]

    X = np.asarray(X, dtype=np.float32)
    in_maps = []
    for c in range(8):
        b, g = divmod(c, 2)
        sl = slice(HD * g, HD * (g + 1))
        in_maps.append({
            "x": np.ascontiguousarray(X[b]),
            "wq": np.ascontiguousarray(np.asarray(Wq, np.float32)[:, sl]),
            "wk": np.ascontiguousarray(np.asarray(Wk, np.float32)[:, sl]),
            "wv": np.ascontiguousarray(np.asarray(Wv, np.float32)[:, sl]),
            "wo": np.ascontiguousarray(np.asarray(Wo, np.float32)[sl, :]),
        })
    res = bass_utils.run_bass_kernel_spmd(nc, in_maps, core_ids=list(range(8)))
    _cache["last_result"] = res
    outf = np.empty((4, T, C), np.float32)
    bo = np.asarray(bo, np.float32)
    for b in range(4):
        outf[b] = res.results[2 * b]["out"] + res.results[2 * b + 1]["out"] + bo
    return outf


# revision 6
# speedup vs baseline: 1.0367x; 1.0009x over previous
"""Multi-head self-attention (B=4, T=2048, C=768, H=12) on 8 trn2 NeuronCores. v9.

Sharding: core c -> batch b=c//2, head-group g=c%2 (6 heads each).
Host sums the 2 partials per batch and adds the bias.

Design:
  - bf16 operands on the PE (FWL weight loads, hidden LDWEIGHTS, 2x DVE modes).
  - head-PAIR row-tiled scores: heads (2m, 2m+1) occupy partition halves of
    qT[m]/kT[m]; score matmuls at tile_position (0,0)/(64,0) run concurrently.
  - v computed directly in [t, d] layout into per-(pair, chunk, head) v_aug
    tiles; a ones column rides along so softmax denominators fall out of the
    ctx matmul.
  - exp mostly on ACT ([128,1024] chunks, bf16 out); ~17% of chunks offloaded
    to DVE via a Schraudolph bf16 exp (tensor_scalar -> int16 bits == bf16).
  - first block's scores+exp are emitted before the v projections (deep pt
    pool buffers 16 chunks) so the ACT stream starts ~25us in; its ctx half
    runs after v lands. Later pairs' projections are emitted mid-attention to
    fill PE gaps.
  - softmax normalize: one [65,1024] copy releases the cps bank; recip +
    gpsimd partition_broadcast + scales run from SBUF off the critical path.

PSUM: lead pool 2 (transposes+projections share slots) + sps 2x2 + cps 2 = 8.
"""
import sys
import os

sys.path.insert(0, "/opt/trn_rl_repo")

import numpy as np

P = 128
T = 2048
C = 768
HD = 384          # per-core head columns (6 heads x 64)
D = 64
NT = T // P       # 16 key chunks of 128
KC = C // P       # 6 contraction chunks for C
MC = HD // P      # 3 head pairs
QB = 512          # query block
NQ = T // QB      # 4 query blocks

EXP_A = 128 * 0.125 * float(np.log2(np.e))   # 23.083120654223414
EXP_B = 128 * 127 - 5.5                      # 16250.5 (Schraudolph bias, bf16)

_cache = {}


def _build(repeat=1):
    import concourse.bacc as bacc
    import concourse.mybir as mybir
    import concourse.tile as tile
    from concourse.masks import make_identity
    from contextlib import ExitStack

    F32 = mybir.dt.float32
    BF16 = mybir.dt.bfloat16
    I16 = mybir.dt.int16
    AF = mybir.ActivationFunctionType
    ALU = mybir.AluOpType

    nc = bacc.Bacc("TRN2", target_bir_lowering=False, debug=False)
    x = nc.dram_tensor("x", [T, C], F32, kind="ExternalInput").ap()
    wq = nc.dram_tensor("wq", [C, HD], F32, kind="ExternalInput").ap()
    wk = nc.dram_tensor("wk", [C, HD], F32, kind="ExternalInput").ap()
    wv = nc.dram_tensor("wv", [C, HD], F32, kind="ExternalInput").ap()
    wo = nc.dram_tensor("wo", [HD, C], F32, kind="ExternalInput").ap()
    out = nc.dram_tensor("out", [T, C], F32, kind="ExternalOutput").ap()

    def emit(pfx, tc, pools):
        ident_bf, big, wrp, vap, work, outp, norm = pools

        xt = [big.tile([P, T], BF16, name=f"{pfx}xt{kc}", tag="big2048") for kc in range(KC)]
        qT = [big.tile([P, T], BF16, name=f"{pfx}qT{m}", tag="big2048") for m in range(MC)]
        kT = [big.tile([P, T], BF16, name=f"{pfx}kT{m}", tag="big2048") for m in range(MC)]
        ctxT = [big.tile([P, T], BF16, name=f"{pfx}ctxT{m}", tag="big2048") for m in range(MC)]
        va = [[[vap.tile([P, D + 1], BF16, name=f"{pfx}va{m}_{t}_{g}", tag=f"va{m}_{t}_{g}")
                for g in range(2)] for t in range(NT)] for m in range(MC)]

        w_b = {}
        wo_b = []

        def emit_weight_loads(wstage, names):
            for nm, src in names:
                if nm == "o":
                    for m in range(MC):
                        st = wstage.tile([P, C], F32, name=f"{pfx}wst_o{m}", tag="wsto")
                        nc.sync.dma_start(st[:], wo[P * m:P * (m + 1), :])
                        t_b = wrp.tile([P, C], BF16, name=f"{pfx}wo_{m}", tag=f"wo_{m}")
                        nc.vector.tensor_copy(t_b[:], st[:])
                        wo_b.append(t_b)
                    continue
                for kc in range(KC):
                    st = wstage.tile([P, HD], F32, name=f"{pfx}wst_{nm}{kc}", tag="wst")
                    nc.sync.dma_start(st[:], src[P * kc:P * (kc + 1), :])
                    t_b = wrp.tile([P, HD], BF16, name=f"{pfx}w_{nm}{kc}", tag=f"w_{nm}{kc}")
                    nc.vector.tensor_copy(t_b[:], st[:])
                    w_b[nm, kc] = t_b

        def emit_xt_group(tq, xrp, xbp, lead):
            xbs = []
            for i in range(4):
                t_i = 4 * tq + i
                xr = xrp.tile([P, C], F32, name=f"{pfx}xr{t_i}", tag="xr")
                nc.sync.dma_start(xr[:], x[P * t_i:P * (t_i + 1), :])
                xb = xbp.tile([P, C], BF16, name=f"{pfx}xb{t_i}", tag="xb")
                nc.vector.tensor_copy(xb[:], xr[:])
                xbs.append(xb)
            for kc in range(KC):
                tp = lead.tile([P, 512], BF16, name=f"{pfx}tp_{tq}_{kc}", tag="lead")
                for i in range(4):
                    nc.tensor.transpose(tp[:, P * i:P * (i + 1)],
                                        xbs[i][:, P * kc:P * (kc + 1)], ident_bf[:])
                nc.vector.tensor_copy(xt[kc][:, 512 * tq:512 * (tq + 1)], tp[:])

        def emit_proj_block(nm, m, n, lead):
            dest = qT if nm == "q" else kT
            ps = lead.tile([P, 512], F32, name=f"{pfx}ps_{nm}{m}{n}", tag="lead")
            for kc in range(KC):
                nc.tensor.matmul(
                    ps[:],
                    w_b[nm, kc][:, P * m:P * (m + 1)],
                    xt[kc][:, 512 * n:512 * (n + 1)],
                    start=(kc == 0), stop=(kc == KC - 1),
                )
            nc.vector.tensor_copy(dest[m][:, 512 * n:512 * (n + 1)], ps[:])

        def emit_v_chunk(t_i, lead):
            pv = lead.tile([P, HD], F32, name=f"{pfx}pv{t_i}", tag="lead")
            for kc in range(KC):
                nc.tensor.matmul(
                    pv[:],
                    xt[kc][:, P * t_i:P * (t_i + 1)],
                    w_b["v", kc][:],
                    start=(kc == 0), stop=(kc == KC - 1),
                )
            for m in range(MC):
                for g in range(2):
                    vt = va[m][t_i][g]
                    nc.vector.tensor_copy(vt[:, 0:D], pv[:, P * m + D * g:P * m + D * (g + 1)])
                    nc.gpsimd.memset(vt[:, D:D + 1], 1.0)

        def scores_exp(m, u, j, spsp):
            q0 = QB * u
            sps = spsp.tile([P, 2 * QB], F32, name=f"{pfx}sps{m}{u}{j}", tag="sps")
            nc.tensor.matmul(sps[:, 0:QB],
                             kT[m][0:D, P * j:P * (j + 1)],
                             qT[m][0:D, q0:q0 + QB],
                             start=True, stop=True, tile_position=(0, 0))
            nc.tensor.matmul(sps[:, QB:2 * QB],
                             kT[m][D:P, P * j:P * (j + 1)],
                             qT[m][D:P, q0:q0 + QB],
                             start=True, stop=True, tile_position=(64, 0))
            pt = work.tile([P, 2 * QB], BF16, name=f"{pfx}pt{m}{u}{j}", tag="pt")
            if (m, u) != (0, 0) and (u * NT + j) % 4 == 3 and 3 <= j <= 12:
                nc.vector.tensor_scalar(pt[:].bitcast(I16), sps[:],
                                        EXP_A, EXP_B, ALU.mult, ALU.add)
            else:
                nc.scalar.activation(pt[:], sps[:], AF.Exp, scale=float(D) ** -0.5)
            return pt

        def ctx_accum(m, u, j, pt, cps):
            for g in range(2):
                nc.tensor.matmul(cps[:, QB * g:QB * (g + 1)],
                                 va[m][j][g][:],
                                 pt[:, QB * g:QB * (g + 1)],
                                 start=(j == 0), stop=(j == NT - 1))

        def normalize(m, u, cps):
            q0 = QB * u
            cu = norm.tile([D + 1, 2 * QB], F32, name=f"{pfx}cu{m}{u}", tag="cu")
            nc.vector.tensor_copy(cu[:], cps[:])
            s_sb = norm.tile([1, 2 * QB], F32, name=f"{pfx}ssb{m}{u}", tag="ssb")
            nc.vector.tensor_copy(s_sb[:], cu[D:D + 1, :])
            rr = norm.tile([1, 2 * QB], F32, name=f"{pfx}rr{m}{u}", tag="rr")
            nc.vector.reciprocal_approx_fast(rr[:], s_sb[:])
            rb = norm.tile([D, 2 * QB], F32, name=f"{pfx}rb{m}{u}", tag="rb")
            nc.gpsimd.partition_broadcast(rb[:], rr[:])
            nc.vector.tensor_mul(ctxT[m][0:D, q0:q0 + QB], cu[0:D, 0:QB], rb[:, 0:QB])
            nc.vector.tensor_mul(ctxT[m][D:P, q0:q0 + QB], cu[0:D, QB:2 * QB], rb[:, QB:2 * QB])

        def attn_block(m, u, spsp, cpsp):
            cps = cpsp.tile([D + 1, 2 * QB], F32, name=f"{pfx}cps{m}_{u}", tag="cps")
            for j in range(NT):
                pt = scores_exp(m, u, j, spsp)
                ctx_accum(m, u, j, pt, cps)
            normalize(m, u, cps)

        def outproj(u, lead):
            for t_i in range(4 * u, 4 * (u + 1)):
                psA = lead.tile([P, 512], F32, name=f"{pfx}psA{t_i}", tag="lead")
                psB = lead.tile([P, C - 512], F32, name=f"{pfx}psB{t_i}", tag="lead")
                for m in range(MC):
                    nc.tensor.matmul(psA[:], ctxT[m][:, P * t_i:P * (t_i + 1)],
                                     wo_b[m][:, 0:512], start=(m == 0), stop=(m == MC - 1))
                    nc.tensor.matmul(psB[:], ctxT[m][:, P * t_i:P * (t_i + 1)],
                                     wo_b[m][:, 512:C], start=(m == 0), stop=(m == MC - 1))
                ob = outp.tile([P, C], F32, name=f"{pfx}ob{t_i}", tag="ob")
                nc.vector.tensor_copy(ob[:, 0:512], psA[:])
                nc.vector.tensor_copy(ob[:, 512:C], psB[:])
                nc.sync.dma_start(out[P * t_i:P * (t_i + 1), :], ob[:])

        # ================= emission =================
        with tc.tile_pool(name=pfx + "xrp", bufs=4) as xrp, \
             tc.tile_pool(name=pfx + "xbp", bufs=5) as xbp, \
             tc.tile_pool(name=pfx + "wstage", bufs=3) as wstage, \
             tc.tile_pool(name=pfx + "lead", bufs=2, space="PSUM") as lead, \
             tc.tile_pool(name=pfx + "sps", bufs=2, space="PSUM") as spsp, \
             tc.tile_pool(name=pfx + "cps", bufs=1, space="PSUM") as cpsp:
            # first block's scores+exp interleave with the lead: exp stream
            # starts as soon as kT[0] n=0 / qT[0] n=0 land (~13us)
            emit_xt_group(0, xrp, xbp, lead)
            emit_weight_loads(wstage, [("k", wk), ("q", wq), ("v", wv)])
            cps00 = cpsp.tile([D + 1, 2 * QB], F32, name=f"{pfx}cps0_0", tag="cps")
            pts00 = []
            emit_proj_block("k", 0, 0, lead)
            emit_proj_block("q", 0, 0, lead)
            for j in range(4):
                pts00.append(scores_exp(0, 0, j, spsp))
            for tq in range(1, 4):
                emit_xt_group(tq, xrp, xbp, lead)
                emit_proj_block("k", 0, tq, lead)
                emit_proj_block("q", 0, tq, lead)
                for j in range(4 * tq, 4 * (tq + 1)):
                    pts00.append(scores_exp(0, 0, j, spsp))
            for t_i in range(NT):
                emit_v_chunk(t_i, lead)
            emit_weight_loads(wstage, [("o", wo)])
            for j in range(NT):
                ctx_accum(0, 0, j, pts00[j], cps00)
            normalize(0, 0, cps00)
            attn_block(0, 1, spsp, cpsp)
            attn_block(0, 2, spsp, cpsp)
            attn_block(0, 3, spsp, cpsp)
            # next pair's projections sit just before their consumers so they
            # only fill attention stalls instead of preempting the exp feed
            for n in range(4):
                emit_proj_block("k", 1, n, lead)
                emit_proj_block("q", 1, n, lead)
            for u in range(NQ):
                attn_block(1, u, spsp, cpsp)
                if u == NQ - 1:
                    for n in range(4):
                        emit_proj_block("k", 2, n, lead)
                        emit_proj_block("q", 2, n, lead)
            for u in range(NQ):
                attn_block(2, u, spsp, cpsp)
                if u >= 1:
                    outproj(u - 1, lead)
            outproj(NQ - 1, lead)

    with tile.TileContext(nc) as tc, ExitStack() as ctx:
        consts = ctx.enter_context(tc.tile_pool(name="consts", bufs=1))
        ident_f32 = consts.tile([P, P], mybir.dt.float32)
        make_identity(nc, ident_f32)
        ident_bf = consts.tile([P, P], BF16)
        nc.vector.tensor_copy(ident_bf[:], ident_f32[:])

        big = ctx.enter_context(tc.tile_pool(name="big", bufs=12))
        wrp = ctx.enter_context(tc.tile_pool(name="wrp", bufs=1))
        vap = ctx.enter_context(tc.tile_pool(name="vap", bufs=1))
        work = ctx.enter_context(tc.tile_pool(name="work", bufs=20))
        outp = ctx.enter_context(tc.tile_pool(name="outp", bufs=2))
        norm = ctx.enter_context(tc.tile_pool(name="norm", bufs=2))
        pools = (ident_bf, big, wrp, vap, work, outp, norm)
        for rep in range(repeat):
            emit(f"r{rep}_", tc, pools)

    nc.compile()
    return nc


def kernel(X, Wq, Wk, Wv, Wo, bo):
    from concourse import bass_utils

    if "nc" not in _cache:
        _cache["nc"] = _build(int(os.environ.get("KERNEL_REPEAT", "1")))
    nc = _cache["nc"]

    X = np.asarray(X, dtype=np.float32)
    in_maps = []
    for c in range(8):
        b, g = divmod(c, 2)
        sl = slice(HD * g, HD * (g + 1))
        in_maps.append({
            "x": np.ascontiguousarray(X[b]),
            "wq": np.ascontiguousarray(np.asarray(Wq, np.float32)[:, sl]),
            "wk": np.ascontiguousarray(np.asarray(Wk, np.float32)[:, sl]),
            "wv": np.ascontiguousarray(np.asarray(Wv, np.float32)[:, sl]),
            "wo": np.ascontiguousarray(np.asarray(Wo, np.float32)[sl, :]),
        })
    res = bass_utils.run_bass_kernel_spmd(nc, in_maps, core_ids=list(range(8)))
    _cache["last_result"] = res
    outf = np.empty((4, T, C), np.float32)
    bo = np.asarray(bo, np.float32)
    for b in range(4):
        outf[b] = res.results[2 * b]["out"] + res.results[2 * b + 1]["out"] + bo
    return outf
